# revision 11
# baseline (speedup 1.0000x reference)
"""Trainium2 Bass kernel for nn_Conv1d_NN (kNN + strided conv).

Math (per batch b):
    dist[t,s]  = ||x[:,t]||^2 + ||x[:,s]||^2 - 2 x[:,t].x[:,s]
    idx[t,:]   = top-8 smallest dist (self first), sorted ascending
    out[o,t]   = sum_{j,c} w[o,c,j] * x[c, idx[t,j]] + b[o]

Device strategy (data-parallel, 2 batches per core on 8 cores):
  - Exact-grade scores via an fp16 split x = h + l (h=fp16(x),
    l=fp16(x-h)) and TWO 1-cycle/row fp16 matmuls per 512-chunk
    accumulated in PSUM fp32:
      pass A: [2h; 1; 1; (100-norm_t)]^T [h; -nhi; -nlo; 1]
      pass B: [2l; 2h]^T [h; l]
    giving S = 100 - dist + O(1e-5)  (the dropped 2*l.l term is ~1e-6).
    2 cycles/row vs fp32's effective 8 -> 4x tensor speedup.
  - DVE drains PSUM directly with a 16->1 group-max tensor_reduce
    (fp32 throughout), then one MAX8 + FIND_INDEX8 over the 128 group
    maxes. A true top-8 column's group always ranks <=8 among groups,
    so the top-8 groups (x16 = 128 candidate columns) cover the true
    top-8; selection noise is fp32-grade (validated rel err ~3e-4).
  - Scores PSUM is two [128,1024] half-tiles (pool bufs=3) so tensor
    matmuls of row tile rt+1 overlap the DVE drain of rt.
  - Conv taps y[t,(j,o)] = sum_c h[c,t] w[o,c,j] + b[o]/8 via one fp16
    matmul per row tile (weights prehalved to pair with the 2h rows);
    scalar converts conv PSUM to fp16 for DMA.
  - Device outputs: y (all taps, fp16) + top-8 group indices (u16).

Host side: exact fp32 re-rank of the 128 candidate columns per token
(this container's runtime has no working data-dependent DMA, so the
rank-indexed gather+sum over the device-computed y/idx tensors runs
host-side, as in the baseline).
"""

import sys
import numpy as np

if "/opt/trn_rl_repo" not in sys.path:
    sys.path.insert(0, "/opt/trn_rl_repo")

B, C, T, K, OUT_C = 16, 64, 2048, 8, 64
NCORES = 8
BPC = B // NCORES  # batches per core
RT = T // 128      # 16 row tiles of 128 contiguous tokens
G = 32             # group size for the DVE pre-reduce
NG = T // G        # 128 groups per row

_CACHE = {}


def build_nc():
    import concourse.bacc as bacc
    import concourse.tile as tile
    import concourse.mybir as mybir

    dt = mybir.dt
    f32 = dt.float32
    f16 = dt.float16
    X = mybir.AxisListType.X
    MAX = mybir.AluOpType.max

    nc = bacc.Bacc(
        "TRN2", target_bir_lowering=False, debug=False, num_devices=NCORES
    )
    rb_d = nc.dram_tensor("rb", [BPC, 128, T], f16, kind="ExternalInput").ap()
    ex_d = nc.dram_tensor("ext", [BPC, 6, T], f16, kind="ExternalInput").ap()
    wall_d = nc.dram_tensor("wall", [65, K * OUT_C], f16, kind="ExternalInput").ap()
    y_d = nc.dram_tensor(
        "yout", [BPC, 128, RT * K * OUT_C], f16, kind="ExternalOutput"
    ).ap()
    g_d = nc.dram_tensor("gidx", [BPC, 128, RT * 8], dt.uint16, kind="ExternalOutput").ap()

    with tile.TileContext(nc) as tc:
        with (
            tc.tile_pool(name="const", bufs=1) as constp,
            tc.tile_pool(name="xio", bufs=2) as xio,
            tc.tile_pool(name="gr", bufs=3) as grp,
            tc.tile_pool(name="small", bufs=2) as smp,
            tc.tile_pool(name="yio", bufs=3) as yp,
            tc.tile_pool(name="pd", bufs=3, space="PSUM") as pdp,
            tc.tile_pool(name="py", bufs=2, space="PSUM") as pyp,
        ):
            wall_sb = constp.tile([65, K * OUT_C], f16)
            nc.sync.dma_start(wall_sb[:], wall_d[:])

            for b in range(BPC):
                la = xio.tile([67, T], f16, tag="la", name=f"la{b}")
                ra = xio.tile([67, T], f16, tag="ra", name=f"ra{b}")
                lb = xio.tile([128, T], f16, tag="lb", name=f"lb{b}")
                rb = xio.tile([128, T], f16, tag="rb", name=f"rb{b}")
                # Only rb = [h; l] and 6 extra rows come over DMA (chunked, in
                # consumption order); la/ra/lb are derived on the otherwise
                # idle GpSimd engine (x2 scales are exact in fp16).
                nc.scalar.dma_start(la[64:67, :], ex_d[b][0:3])
                nc.scalar.dma_start(ra[64:67, :], ex_d[b][3:6])
                for q in range(4):
                    qs = slice(q * 512, (q + 1) * 512)
                    nc.sync.dma_start(rb[:, qs], rb_d[b][:, qs])
                    nc.gpsimd.tensor_scalar_mul(la[0:64, qs], rb[0:64, qs], 2.0)
                    nc.gpsimd.tensor_scalar_mul(ra[0:64, qs], rb[0:64, qs], 1.0)
                    nc.gpsimd.tensor_scalar_mul(lb[0:64, qs], rb[64:128, qs], 2.0)
                    nc.gpsimd.tensor_scalar_mul(lb[64:128, qs], rb[0:64, qs], 2.0)

                gall = smp.tile([128, RT * 8], dt.uint16, tag="gall", name=f"gall{b}")

                pend = None  # (vals, gr) of the previous row tile
                for rt in range(RT):
                    tsl = slice(rt * 128, (rt + 1) * 128)
                    gr = grp.tile([128, NG], f32, tag="gr", name=f"gr{b}_{rt}")
                    for h in range(2):
                        psH = pdp.tile(
                            [128, 1024], f32, tag="pd", name=f"pd{b}_{rt}_{h}"
                        )
                        for q in range(2):
                            ssl = slice(h * 1024 + q * 512, h * 1024 + (q + 1) * 512)
                            osl = slice(q * 512, (q + 1) * 512)
                            nc.tensor.matmul(
                                psH[:, osl], la[:, tsl], ra[:, ssl],
                                start=True, stop=False,
                            )
                            nc.tensor.matmul(
                                psH[:, osl], lb[:, tsl], rb[:, ssl],
                                start=False, stop=True,
                            )
                        nc.vector.tensor_reduce(
                            gr[:, h * (NG // 2) : (h + 1) * (NG // 2)],
                            psH.rearrange("p (g k) -> p g k", k=G),
                            X,
                            MAX,
                        )
                    psY = pyp.tile([128, K * OUT_C], f32, tag="py", name=f"py{b}_{rt}")
                    nc.tensor.matmul(psY[:], la[0:65, tsl], wall_sb[:])

                    vals = smp.tile([128, 8], f32, tag="vals", name=f"v{b}_{rt}")
                    nc.vector.max(vals[:], gr[:])
                    # max_index of the previous row tile is emitted here, one
                    # tile late: its (coarsened) cross-engine wait is then
                    # already satisfied, keeping the DVE queue head unblocked.
                    if pend is not None:
                        pvals, pgr, prt = pend
                        nc.vector.max_index(
                            gall[:, prt * 8 : (prt + 1) * 8], pvals[:], pgr[:]
                        )
                    pend = (vals, gr, rt)

                    if rt % 4 == 0:
                        ybig = yp.tile(
                            [128, 4 * K * OUT_C], f16, tag="y16", name=f"y{b}_{rt}"
                        )
                    nc.scalar.copy(
                        y16s := ybig[:, (rt % 4) * 512 : (rt % 4 + 1) * 512], psY[:]
                    )
                    if rt % 4 == 3:
                        nc.sync.dma_start(
                            y_d[b][:, (rt - 3) * 512 : (rt + 1) * 512], ybig[:]
                        )

                pvals, pgr, prt = pend
                nc.vector.max_index(
                    gall[:, prt * 8 : (prt + 1) * 8], pvals[:], pgr[:]
                )
                nc.sync.dma_start(g_d[b], gall[:])

    nc.compile()
    return nc


def _get_nc():
    if "nc" not in _CACHE:
        _CACHE["nc"] = build_nc()
    return _CACHE["nc"]


def host_inputs(x, w, b):
    """Per-core input maps from full inputs."""
    x = np.asarray(x, dtype=np.float32)
    w = np.asarray(w, dtype=np.float32)
    b = np.asarray(b, dtype=np.float32)

    h = x.astype(np.float16)                                   # [B, C, T]
    l = (x.astype(np.float64) - h.astype(np.float64)).astype(np.float16)
    norm = np.sum(x.astype(np.float64) * x.astype(np.float64), axis=1)  # [B, T]
    nhi = norm.astype(np.float16)
    nlo = (norm - nhi.astype(np.float64)).astype(np.float16)
    cent = (100.0 - norm).astype(np.float16)
    ones = np.ones((B, 1, T), np.float16)

    rb = np.concatenate([h, l], axis=1)          # [B, 128, T]
    ext = np.concatenate(
        [ones, ones, cent[:, None, :], -nhi[:, None, :], -nlo[:, None, :], ones],
        axis=1,
    )  # [B, 6, T]

    wall = np.empty((65, K * OUT_C), np.float16)
    wall[:C] = (w.transpose(1, 2, 0).reshape(C, K * OUT_C) / 2).astype(np.float16)
    wall[C] = np.tile((b / K).astype(np.float16), K)

    return [
        {
            "rb": np.ascontiguousarray(rb[i * BPC : (i + 1) * BPC]),
            "ext": np.ascontiguousarray(ext[i * BPC : (i + 1) * BPC]),
            "wall": wall,
        }
        for i in range(NCORES)
    ]


def kernel(x, w, b):
    from concourse.bass_utils import run_bass_kernel_spmd

    x = np.asarray(x, dtype=np.float32)
    nc = _get_nc()
    in_maps = host_inputs(x, w, b)
    res = run_bass_kernel_spmd(nc, in_maps, list(range(NCORES)))

    out = np.empty((B, OUT_C, T), np.float32)
    jj = np.arange(K)[None, :]
    goff = np.arange(G)[None, None, :]
    for i in range(NCORES):
        yv = res.results[i]["yout"]   # [BPC, 128, RT*512] f16
        gi = res.results[i]["gidx"]   # [BPC, 128, RT*8] u16
        for bb in range(BPC):
            gb = i * BPC + bb
            # token t = rt*128 + p  ->  groups at gi[bb][p, rt*8 + j]
            g = gi[bb].reshape(128, RT, 8).transpose(1, 0, 2).reshape(T, 8)
            g = np.minimum(g.astype(np.int64), NG - 1)
            cand = (g[:, :, None] * G + goff).reshape(T, 8 * G)   # [T, 128]
            cand = np.sort(cand, axis=1)
            # exact re-rank (fp32, same formula as reference)
            xb = x[gb]                                  # [C, T]
            nb = np.sum(xb * xb, axis=0)                # [T]
            dots = np.einsum("ct,ctk->tk", xb, xb[:, cand])
            dist = nb[:, None] + nb[cand] - 2 * dots
            order = np.argsort(dist, axis=1, kind="stable")[:, :K]
            sel = np.take_along_axis(cand, order, axis=1)         # [T, K]
            # gather-sum the device conv taps (token t = rt*128 + p is
            # stored at yv[bb][p, rt*512:(rt+1)*512])
            yb = (
                yv[bb].reshape(128, RT, K * OUT_C).transpose(1, 0, 2)
                .reshape(T, K, OUT_C).astype(np.float32)
            )
            gath = yb[sel, jj, :]                                 # [T, K, OUT_C]
            out[gb] = gath.sum(1).T
    return out.astype(np.float32)


# revision 12
# speedup vs baseline: 1.7972x; 1.7972x over previous
"""Trainium2 Bass kernel for nn_Conv1d_NN (kNN + strided conv).

Math (per batch b):
    dist[t,s]  = ||x[:,t]||^2 + ||x[:,s]||^2 - 2 x[:,t].x[:,s]
    idx[t,:]   = top-8 smallest dist (self first), sorted ascending
    out[o,t]   = sum_{j,c} w[o,c,j] * x[c, idx[t,j]] + b[o]

Device strategy (data-parallel, 2 batches per core on 8 cores):
  - Exact-grade scores via an fp16 split x = h + l (h=fp16(x),
    l=fp16(x-h)) and TWO 1-cycle/row fp16 matmuls per 512-chunk
    accumulated in PSUM fp32:
      pass A: [2h; 1; 1; (100-norm_t)]^T [h; -nhi; -nlo; 1]
      pass B: [2l; 2h]^T [h; l]
    giving S = 100 - dist + O(1e-5)  (the dropped 2*l.l term is ~1e-6).
    2 cycles/row vs fp32's effective 8 -> 4x tensor speedup.
  - DVE drains PSUM directly with a 16->1 group-max tensor_reduce
    (fp32 throughout), then one MAX8 + FIND_INDEX8 over the 128 group
    maxes. A true top-8 column's group always ranks <=8 among groups,
    so the top-8 groups (x16 = 128 candidate columns) cover the true
    top-8; selection noise is fp32-grade (validated rel err ~3e-4).
  - Scores PSUM is two [128,1024] half-tiles (pool bufs=3) so tensor
    matmuls of row tile rt+1 overlap the DVE drain of rt.
  - Conv taps y[t,(j,o)] = sum_c h[c,t] w[o,c,j] + b[o]/8 via one fp16
    matmul per row tile (weights prehalved to pair with the 2h rows);
    scalar converts conv PSUM to fp16 for DMA.
  - Device outputs: y (all taps, fp16) + top-8 group indices (u16).

Host side: exact fp32 re-rank of the 128 candidate columns per token
(this container's runtime has no working data-dependent DMA, so the
rank-indexed gather+sum over the device-computed y/idx tensors runs
host-side, as in the baseline).
"""

import sys
import numpy as np

if "/opt/trn_rl_repo" not in sys.path:
    sys.path.insert(0, "/opt/trn_rl_repo")

B, C, T, K, OUT_C = 16, 64, 2048, 8, 64
NCORES = 8
BPC = B // NCORES  # batches per core
RT = T // 128      # 16 row tiles of 128 contiguous tokens
G = 32             # group size for the DVE pre-reduce
NG = T // G        # 128 groups per row

_CACHE = {}


def build_nc():
    import concourse.bacc as bacc
    import concourse.tile as tile
    import concourse.mybir as mybir

    dt = mybir.dt
    f32 = dt.float32
    f16 = dt.float16
    X = mybir.AxisListType.X
    MAX = mybir.AluOpType.max

    nc = bacc.Bacc(
        "TRN2", target_bir_lowering=False, debug=False, num_devices=NCORES
    )
    la_d = nc.dram_tensor("la", [BPC, 67, T], f16, kind="ExternalInput").ap()
    ra_d = nc.dram_tensor("ra", [BPC, 67, T], f16, kind="ExternalInput").ap()
    lb_d = nc.dram_tensor("lb", [BPC, 128, T], f16, kind="ExternalInput").ap()
    rb_d = nc.dram_tensor("rb", [BPC, 128, T], f16, kind="ExternalInput").ap()
    wall_d = nc.dram_tensor("wall", [65, K * OUT_C], f16, kind="ExternalInput").ap()
    y_d = nc.dram_tensor(
        "yout", [BPC, 128, RT * K * OUT_C], f16, kind="ExternalOutput"
    ).ap()
    g_d = nc.dram_tensor("gidx", [BPC, 128, RT * 8], dt.uint16, kind="ExternalOutput").ap()

    with tile.TileContext(nc) as tc:
        with (
            tc.tile_pool(name="const", bufs=1) as constp,
            tc.tile_pool(name="xio", bufs=2) as xio,
            tc.tile_pool(name="gr", bufs=3) as grp,
            tc.tile_pool(name="small", bufs=2) as smp,
            tc.tile_pool(name="yio", bufs=3) as yp,
            tc.tile_pool(name="pd", bufs=3, space="PSUM") as pdp,
            tc.tile_pool(name="py", bufs=2, space="PSUM") as pyp,
        ):
            wall_sb = constp.tile([65, K * OUT_C], f16)
            nc.sync.dma_start(wall_sb[:], wall_d[:])

            for b in range(BPC):
                la = xio.tile([67, T], f16, tag="la", name=f"la{b}")
                ra = xio.tile([67, T], f16, tag="ra", name=f"ra{b}")
                lb = xio.tile([128, T], f16, tag="lb", name=f"lb{b}")
                rb = xio.tile([128, T], f16, tag="rb", name=f"rb{b}")
                # chunked loads in consumption order so row tile 0 can
                # start after ~1/4 of the batch input has landed
                for q in range(4):
                    qs = slice(q * 512, (q + 1) * 512)
                    nc.sync.dma_start(la[:, qs], la_d[b][:, qs])
                    nc.scalar.dma_start(ra[:, qs], ra_d[b][:, qs])
                    nc.sync.dma_start(lb[:, qs], lb_d[b][:, qs])
                    nc.scalar.dma_start(rb[:, qs], rb_d[b][:, qs])

                gall = smp.tile([128, RT * 8], dt.uint16, tag="gall", name=f"gall{b}")

                pend = None  # (vals, gr) of the previous row tile
                for rt in range(RT):
                    tsl = slice(rt * 128, (rt + 1) * 128)
                    gr = grp.tile([128, NG], f32, tag="gr", name=f"gr{b}_{rt}")
                    for h in range(2):
                        psH = pdp.tile(
                            [128, 1024], f32, tag="pd", name=f"pd{b}_{rt}_{h}"
                        )
                        for q in range(2):
                            ssl = slice(h * 1024 + q * 512, h * 1024 + (q + 1) * 512)
                            osl = slice(q * 512, (q + 1) * 512)
                            nc.tensor.matmul(
                                psH[:, osl], la[:, tsl], ra[:, ssl],
                                start=True, stop=False,
                            )
                            nc.tensor.matmul(
                                psH[:, osl], lb[:, tsl], rb[:, ssl],
                                start=False, stop=True,
                            )
                        nc.vector.tensor_reduce(
                            gr[:, h * (NG // 2) : (h + 1) * (NG // 2)],
                            psH.rearrange("p (g k) -> p g k", k=G),
                            X,
                            MAX,
                        )
                    psY = pyp.tile([128, K * OUT_C], f32, tag="py", name=f"py{b}_{rt}")
                    nc.tensor.matmul(psY[:], la[0:65, tsl], wall_sb[:])

                    vals = smp.tile([128, 8], f32, tag="vals", name=f"v{b}_{rt}")
                    nc.vector.max(vals[:], gr[:])
                    # max_index of the previous row tile is emitted here, one
                    # tile late: its (coarsened) cross-engine wait is then
                    # already satisfied, keeping the DVE queue head unblocked.
                    if pend is not None:
                        pvals, pgr, prt = pend
                        nc.vector.max_index(
                            gall[:, prt * 8 : (prt + 1) * 8], pvals[:], pgr[:]
                        )
                    pend = (vals, gr, rt)

                    if rt % 4 == 0:
                        ybig = yp.tile(
                            [128, 4 * K * OUT_C], f16, tag="y16", name=f"y{b}_{rt}"
                        )
                    nc.scalar.copy(
                        y16s := ybig[:, (rt % 4) * 512 : (rt % 4 + 1) * 512], psY[:]
                    )
                    if rt % 4 == 3:
                        nc.sync.dma_start(
                            y_d[b][:, (rt - 3) * 512 : (rt + 1) * 512], ybig[:]
                        )

                pvals, pgr, prt = pend
                nc.vector.max_index(
                    gall[:, prt * 8 : (prt + 1) * 8], pvals[:], pgr[:]
                )
                nc.sync.dma_start(g_d[b], gall[:])

    nc.compile()
    return nc


def _get_nc():
    if "nc" not in _CACHE:
        _CACHE["nc"] = build_nc()
    return _CACHE["nc"]


def host_inputs(x, w, b):
    """Per-core input maps from full inputs."""
    x = np.asarray(x, dtype=np.float32)
    w = np.asarray(w, dtype=np.float32)
    b = np.asarray(b, dtype=np.float32)

    h = x.astype(np.float16)                                   # [B, C, T]
    l = (x.astype(np.float64) - h.astype(np.float64)).astype(np.float16)
    norm = np.sum(x.astype(np.float64) * x.astype(np.float64), axis=1)  # [B, T]
    nhi = norm.astype(np.float16)
    nlo = (norm - nhi.astype(np.float64)).astype(np.float16)
    cent = (100.0 - norm).astype(np.float16)
    ones = np.ones((B, 1, T), np.float16)

    la = np.concatenate(
        [2 * h, ones, ones, cent[:, None, :]], axis=1
    )  # [B, 67, T]
    ra = np.concatenate(
        [h, -nhi[:, None, :], -nlo[:, None, :], ones], axis=1
    )  # [B, 67, T]
    lb = np.concatenate([2 * l, 2 * h], axis=1)  # [B, 128, T]
    rb = np.concatenate([h, l], axis=1)          # [B, 128, T]

    wall = np.empty((65, K * OUT_C), np.float16)
    wall[:C] = (w.transpose(1, 2, 0).reshape(C, K * OUT_C) / 2).astype(np.float16)
    wall[C] = np.tile((b / K).astype(np.float16), K)

    return [
        {
            "la": np.ascontiguousarray(la[i * BPC : (i + 1) * BPC]),
            "ra": np.ascontiguousarray(ra[i * BPC : (i + 1) * BPC]),
            "lb": np.ascontiguousarray(lb[i * BPC : (i + 1) * BPC]),
            "rb": np.ascontiguousarray(rb[i * BPC : (i + 1) * BPC]),
            "wall": wall,
        }
        for i in range(NCORES)
    ]


def kernel(x, w, b):
    from concourse.bass_utils import run_bass_kernel_spmd

    x = np.asarray(x, dtype=np.float32)
    nc = _get_nc()
    in_maps = host_inputs(x, w, b)
    res = run_bass_kernel_spmd(nc, in_maps, list(range(NCORES)))

    out = np.empty((B, OUT_C, T), np.float32)
    jj = np.arange(K)[None, :]
    goff = np.arange(G)[None, None, :]
    for i in range(NCORES):
        yv = res.results[i]["yout"]   # [BPC, 128, RT*512] f16
        gi = res.results[i]["gidx"]   # [BPC, 128, RT*8] u16
        for bb in range(BPC):
            gb = i * BPC + bb
            # token t = rt*128 + p  ->  groups at gi[bb][p, rt*8 + j]
            g = gi[bb].reshape(128, RT, 8).transpose(1, 0, 2).reshape(T, 8)
            g = np.minimum(g.astype(np.int64), NG - 1)
            cand = (g[:, :, None] * G + goff).reshape(T, 8 * G)   # [T, 128]
            cand = np.sort(cand, axis=1)
            # exact re-rank (fp32, same formula as reference)
            xb = x[gb]                                  # [C, T]
            nb = np.sum(xb * xb, axis=0)                # [T]
            dots = np.einsum("ct,ctk->tk", xb, xb[:, cand])
            dist = nb[:, None] + nb[cand] - 2 * dots
            order = np.argsort(dist, axis=1, kind="stable")[:, :K]
            sel = np.take_along_axis(cand, order, axis=1)         # [T, K]
            # gather-sum the device conv taps (token t = rt*128 + p is
            # stored at yv[bb][p, rt*512:(rt+1)*512])
            yb = (
                yv[bb].reshape(128, RT, K * OUT_C).transpose(1, 0, 2)
                .reshape(T, K, OUT_C).astype(np.float32)
            )
            gath = yb[sel, jj, :]                                 # [T, K, OUT_C]
            out[gb] = gath.sum(1).T
    return out.astype(np.float32)


# revision 13
# speedup vs baseline: 2.3980x; 1.3343x over previous
"""Trainium2 Bass kernel for nn_Conv1d_NN (kNN + strided conv).

Math (per batch b):
    dist[t,s]  = ||x[:,t]||^2 + ||x[:,s]||^2 - 2 x[:,t].x[:,s]
    idx[t,:]   = top-8 smallest dist (self first), sorted ascending
    out[o,t]   = sum_{j,c} w[o,c,j] * x[c, idx[t,j]] + b[o]

Device strategy (data-parallel, 2 batches per core on 8 cores):
  - Exact-grade scores via an fp16 split x = h + l (h=fp16(x),
    l=fp16(x-h)) and TWO 1-cycle/row fp16 matmuls per 512-chunk
    accumulated in PSUM fp32:
      pass A: [2h; 1; 1; (100-norm_t)]^T [h; -nhi; -nlo; 1]
      pass B: [2l; 2h]^T [h; l]
    giving S = 100 - dist + O(1e-5)  (the dropped 2*l.l term is ~1e-6).
    2 cycles/row vs fp32's effective 8 -> 4x tensor speedup.
  - DVE drains PSUM directly with a 16->1 group-max tensor_reduce
    (fp32 throughout), then one MAX8 + FIND_INDEX8 over the 128 group
    maxes. A true top-8 column's group always ranks <=8 among groups,
    so the top-8 groups (x16 = 128 candidate columns) cover the true
    top-8; selection noise is fp32-grade (validated rel err ~3e-4).
  - Scores PSUM is two [128,1024] half-tiles (pool bufs=3) so tensor
    matmuls of row tile rt+1 overlap the DVE drain of rt.
  - Conv taps y[t,(j,o)] = sum_c h[c,t] w[o,c,j] + b[o]/8 via one fp16
    matmul per row tile (weights prehalved to pair with the 2h rows);
    scalar converts conv PSUM to fp16 for DMA.
  - Device outputs: y (all taps, fp16) + top-8 group indices (u16).

Host side: exact fp32 re-rank of the 128 candidate columns per token
(this container's runtime has no working data-dependent DMA, so the
rank-indexed gather+sum over the device-computed y/idx tensors runs
host-side, as in the baseline).
"""

import sys
import numpy as np

if "/opt/trn_rl_repo" not in sys.path:
    sys.path.insert(0, "/opt/trn_rl_repo")

B, C, T, K, OUT_C = 16, 64, 2048, 8, 64
NCORES = 8
BPC = B // NCORES  # batches per core
RT = T // 128      # 16 row tiles of 128 contiguous tokens
G = 32             # group size for the DVE pre-reduce
NG = T // G        # 128 groups per row

_CACHE = {}


def build_nc():
    import concourse.bacc as bacc
    import concourse.tile as tile
    import concourse.mybir as mybir

    dt = mybir.dt
    f32 = dt.float32
    f16 = dt.float16
    X = mybir.AxisListType.X
    MAX = mybir.AluOpType.max

    nc = bacc.Bacc(
        "TRN2", target_bir_lowering=False, debug=False, num_devices=NCORES
    )
    ra_d = nc.dram_tensor("ra", [BPC, 67, T], f16, kind="ExternalInput").ap()
    rb_d = nc.dram_tensor("rb", [BPC, 128, T], f16, kind="ExternalInput").ap()
    ex_d = nc.dram_tensor("ext", [BPC, 3, T], f16, kind="ExternalInput").ap()
    wall_d = nc.dram_tensor("wall", [65, K * OUT_C], f16, kind="ExternalInput").ap()
    y_d = nc.dram_tensor(
        "yout", [BPC, 128, RT * K * OUT_C], f16, kind="ExternalOutput"
    ).ap()
    g_d = nc.dram_tensor("gidx", [BPC, 128, RT * 8], dt.uint16, kind="ExternalOutput").ap()

    with tile.TileContext(nc) as tc:
        with (
            tc.tile_pool(name="const", bufs=1) as constp,
            tc.tile_pool(name="xio", bufs=2) as xio,
            tc.tile_pool(name="gr", bufs=3) as grp,
            tc.tile_pool(name="small", bufs=2) as smp,
            tc.tile_pool(name="yio", bufs=3) as yp,
            tc.tile_pool(name="pd", bufs=3, space="PSUM") as pdp,
            tc.tile_pool(name="py", bufs=2, space="PSUM") as pyp,
        ):
            wall_sb = constp.tile([65, K * OUT_C], f16)
            nc.sync.dma_start(wall_sb[:], wall_d[:])

            for b in range(BPC):
                la = xio.tile([67, T], f16, tag="la", name=f"la{b}")
                ra = xio.tile([67, T], f16, tag="ra", name=f"ra{b}")
                lb = xio.tile([128, T], f16, tag="lb", name=f"lb{b}")
                rb = xio.tile([128, T], f16, tag="rb", name=f"rb{b}")
                # Only ra, rb and la's 3 extra rows come over DMA (chunked,
                # in consumption order); la rows 0-63 = 2h and lb = [2l; 2h]
                # are derived from rb on the scalar engine (x2 is exact).
                nc.sync.dma_start(la[64:67, :], ex_d[b])
                for q in range(4):
                    qs = slice(q * 512, (q + 1) * 512)
                    nc.sync.dma_start(rb[:, qs], rb_d[b][:, qs])
                    nc.sync.dma_start(ra[:, qs], ra_d[b][:, qs])
                    nc.scalar.mul(la[0:64, qs], rb[0:64, qs], 2.0)
                    nc.scalar.mul(lb[0:64, qs], rb[64:128, qs], 2.0)
                    nc.scalar.mul(lb[64:128, qs], rb[0:64, qs], 2.0)

                gall = smp.tile([128, RT * 8], dt.uint16, tag="gall", name=f"gall{b}")

                pend = None  # (vals, gr) of the previous row tile
                for rt in range(RT):
                    tsl = slice(rt * 128, (rt + 1) * 128)
                    gr = grp.tile([128, NG], f32, tag="gr", name=f"gr{b}_{rt}")
                    for h in range(2):
                        psH = pdp.tile(
                            [128, 1024], f32, tag="pd", name=f"pd{b}_{rt}_{h}"
                        )
                        for q in range(2):
                            ssl = slice(h * 1024 + q * 512, h * 1024 + (q + 1) * 512)
                            osl = slice(q * 512, (q + 1) * 512)
                            nc.tensor.matmul(
                                psH[:, osl], la[:, tsl], ra[:, ssl],
                                start=True, stop=False,
                            )
                            nc.tensor.matmul(
                                psH[:, osl], lb[:, tsl], rb[:, ssl],
                                start=False, stop=True,
                            )
                        nc.vector.tensor_reduce(
                            gr[:, h * (NG // 2) : (h + 1) * (NG // 2)],
                            psH.rearrange("p (g k) -> p g k", k=G),
                            X,
                            MAX,
                        )
                    psY = pyp.tile([128, K * OUT_C], f32, tag="py", name=f"py{b}_{rt}")
                    nc.tensor.matmul(psY[:], la[0:65, tsl], wall_sb[:])

                    vals = smp.tile([128, 8], f32, tag="vals", name=f"v{b}_{rt}")
                    nc.vector.max(vals[:], gr[:])
                    # max_index of the previous row tile is emitted here, one
                    # tile late: its (coarsened) cross-engine wait is then
                    # already satisfied, keeping the DVE queue head unblocked.
                    if pend is not None:
                        pvals, pgr, prt = pend
                        nc.vector.max_index(
                            gall[:, prt * 8 : (prt + 1) * 8], pvals[:], pgr[:]
                        )
                    pend = (vals, gr, rt)

                    if rt % 4 == 0:
                        ybig = yp.tile(
                            [128, 4 * K * OUT_C], f16, tag="y16", name=f"y{b}_{rt}"
                        )
                    nc.scalar.copy(
                        y16s := ybig[:, (rt % 4) * 512 : (rt % 4 + 1) * 512], psY[:]
                    )
                    if rt % 4 == 3:
                        nc.sync.dma_start(
                            y_d[b][:, (rt - 3) * 512 : (rt + 1) * 512], ybig[:]
                        )

                pvals, pgr, prt = pend
                nc.vector.max_index(
                    gall[:, prt * 8 : (prt + 1) * 8], pvals[:], pgr[:]
                )
                nc.sync.dma_start(g_d[b], gall[:])

    nc.compile()
    return nc


def _get_nc():
    if "nc" not in _CACHE:
        _CACHE["nc"] = build_nc()
    return _CACHE["nc"]


def host_inputs(x, w, b):
    """Per-core input maps from full inputs."""
    x = np.asarray(x, dtype=np.float32)
    w = np.asarray(w, dtype=np.float32)
    b = np.asarray(b, dtype=np.float32)

    h = x.astype(np.float16)                                   # [B, C, T]
    l = (x.astype(np.float64) - h.astype(np.float64)).astype(np.float16)
    norm = np.sum(x.astype(np.float64) * x.astype(np.float64), axis=1)  # [B, T]
    nhi = norm.astype(np.float16)
    nlo = (norm - nhi.astype(np.float64)).astype(np.float16)
    cent = (100.0 - norm).astype(np.float16)
    ones = np.ones((B, 1, T), np.float16)

    ra = np.concatenate(
        [h, -nhi[:, None, :], -nlo[:, None, :], ones], axis=1
    )  # [B, 67, T]
    rb = np.concatenate([h, l], axis=1)          # [B, 128, T]
    ext = np.concatenate([ones, ones, cent[:, None, :]], axis=1)  # [B, 3, T]

    wall = np.empty((65, K * OUT_C), np.float16)
    wall[:C] = (w.transpose(1, 2, 0).reshape(C, K * OUT_C) / 2).astype(np.float16)
    wall[C] = np.tile((b / K).astype(np.float16), K)

    return [
        {
            "ra": np.ascontiguousarray(ra[i * BPC : (i + 1) * BPC]),
            "rb": np.ascontiguousarray(rb[i * BPC : (i + 1) * BPC]),
            "ext": np.ascontiguousarray(ext[i * BPC : (i + 1) * BPC]),
            "wall": wall,
        }
        for i in range(NCORES)
    ]


def kernel(x, w, b):
    from concourse.bass_utils import run_bass_kernel_spmd

    x = np.asarray(x, dtype=np.float32)
    nc = _get_nc()
    in_maps = host_inputs(x, w, b)
    res = run_bass_kernel_spmd(nc, in_maps, list(range(NCORES)))

    out = np.empty((B, OUT_C, T), np.float32)
    jj = np.arange(K)[None, :]
    goff = np.arange(G)[None, None, :]
    for i in range(NCORES):
        yv = res.results[i]["yout"]   # [BPC, 128, RT*512] f16
        gi = res.results[i]["gidx"]   # [BPC, 128, RT*8] u16
        for bb in range(BPC):
            gb = i * BPC + bb
            # token t = rt*128 + p  ->  groups at gi[bb][p, rt*8 + j]
            g = gi[bb].reshape(128, RT, 8).transpose(1, 0, 2).reshape(T, 8)
            g = np.minimum(g.astype(np.int64), NG - 1)
            cand = (g[:, :, None] * G + goff).reshape(T, 8 * G)   # [T, 128]
            cand = np.sort(cand, axis=1)
            # exact re-rank (fp32, same formula as reference)
            xb = x[gb]                                  # [C, T]
            nb = np.sum(xb * xb, axis=0)                # [T]
            dots = np.einsum("ct,ctk->tk", xb, xb[:, cand])
            dist = nb[:, None] + nb[cand] - 2 * dots
            order = np.argsort(dist, axis=1, kind="stable")[:, :K]
            sel = np.take_along_axis(cand, order, axis=1)         # [T, K]
            # gather-sum the device conv taps (token t = rt*128 + p is
            # stored at yv[bb][p, rt*512:(rt+1)*512])
            yb = (
                yv[bb].reshape(128, RT, K * OUT_C).transpose(1, 0, 2)
                .reshape(T, K, OUT_C).astype(np.float32)
            )
            gath = yb[sel, jj, :]                                 # [T, K, OUT_C]
            out[gb] = gath.sum(1).T
    return out.astype(np.float32)
